# revision 28
# baseline (speedup 1.0000x reference)
"""Multi-head attention kernel for Trainium2 (Bass/Tile), 8 NeuronCores.

Problem: nn_MultiHeadAttention
  x [8, 1024, 1024] f32, w_qkv [1024, 3072], b_qkv [3072],
  w_proj [1024, 1024], b_proj [1024]  ->  out [8, 1024, 1024]

  qkv = x @ w_qkv + b_qkv ; split (h, d, 3) interleaved on last dim
  score = q k^T per (b, h);  att = softmax(score, -1) / sqrt(1024)
  out = (att @ v) reshaped @ w_proj + b_proj

Sharding: data-parallel over batch. Each of the 8 cores runs the full
MHA for one batch element; no collectives.

v2 design (post-trace): the v1 kernel ran at 680us with the PE
clock-gated to 1.2GHz for 2/3 of the span (HAM re-throttle during
serial per-pair stalls) and 104us of single-partition DVE RECIPROCAL.
This version:
  - keeps the PE warm: attention is an ACT-paced conveyor (scores ->
    exp -> att@V per k-tile, ping-pong PSUM), with the NEXT pair's
    Q/K projections interleaved into the same span so the PE never
    idles long enough to re-throttle.
  - scores run as two concurrent row-tiled (64x128) matmuls: even
    head on PE tile T0 (SBUF rows 0:64), odd head on T8 (rows 64:128).
  - softmax denominators come free from a 65th "32.0" column in the
    V operand (folds the 1/sqrt(D) post-scale); their reciprocals are
    computed 2 rows at a time with reciprocal_approx_fast (~5x faster,
    128 partitions wide) and broadcast across partitions with one
    K=2 indicator matmul.
  - all weight DMA slices are contiguous per partition (per-pair /
    per-chunk major DRAM layout).

Device-side math per core (no on-device transpose anywhere):
  qT = (x wq)^T  [(h,d), tok]   kT likewise
  v_aug = [x wv + bv | 32.0] per head   [tok, h*(d+1)]
  per head pair, per k-tile: S^T[k,q] = kT.T-slice @ qT-slice (T0/T8)
     E = exp(S^T); O'^T[0:64,q] += v_aug.T @ E; O'^T[64,q] = 32*denom
  ao^T = O' * (1/(32*denom))  (recip via DVE approx, bcast via PE)
  out = ao^T.T @ wp + bp
"""

import os

os.environ.setdefault("MYCRO_LOCAL_CACHE", "1")

import numpy as np

import concourse.bass as bass
import concourse.tile as tile
from concourse import bacc, mybir

P = 128
DH = 64  # head dim
F32 = mybir.dt.float32
F32R = mybir.dt.float32r
BF = mybir.dt.bfloat16
F16 = mybir.dt.float16
# matmul-operand dtype for the score path (x, wq/wk, qT/kT): f32r keeps
# 11 mantissa bits, needed because score errors pass through exp().
# Everything else (V, E, attout, proj) runs bf16: same 1 cycle/col PE
# rate but fast weight loads and half the DMA/SBUF footprint.
MM = F32R

# full-problem constants
B_FULL = 8
TOK_FULL = 1024
D_FULL = 1024
H_FULL = 16
ATT_SCALE_FULL = 1.0 / 32.0  # 1/sqrt(1024), applied after softmax
N_CORES = 8


def build(nc, TOK, D, H, att_scale):
    """Emit the one-core MHA program (one batch element).

    DRAM inputs (host pre-laid-out, all slices contiguous/partition):
      x        [P, KT*TOK]        [p][kt][t] = x[t, kt*P + p]
      wq, wk   [P, NPAIR*KT*P]    [p][pair][kt][n]; cols n = pair block
      wv, wp   [P, NVCH*KT*VCH]   [p][c][kt][n];  cols n = chunk block
      bq, bk   [P, NPAIR]         [p][pair] = b[pair*P + p]
      bv, bp   [1, D]
    Output: out [TOK, D] f32
    """
    assert D == H * DH and D % P == 0 and TOK % P == 0 and H % 2 == 0
    KT = D // P        # contraction tiles over the model dim
    MT = TOK // P      # token tiles (also the k-tiles of attention)
    NPAIR = H // 2     # head pairs
    VW = H * (DH + 1)  # v_aug row width: per head [v | aug]
    QCH = min(512, TOK)   # moving-chunk width for scores / att@V / QK
    NQH = TOK // QCH
    VCH = min(512, D)     # column chunk for V / proj weight streaming
    NVCH = D // VCH
    AUG = 1.0 / att_scale  # 32.0: folded post-softmax scale
    EXP = mybir.ActivationFunctionType.Exp
    assert MT % 2 == 0

    x_d = nc.dram_tensor("x", [P, KT * TOK], F16, kind="ExternalInput")
    wq_d = nc.dram_tensor("wq", [P, NPAIR * KT * P], F16, kind="ExternalInput")
    wk_d = nc.dram_tensor("wk", [P, NPAIR * KT * P], F16, kind="ExternalInput")
    wv_d = nc.dram_tensor("wv", [P, KT * D], F16, kind="ExternalInput")
    wp_d = nc.dram_tensor("wp", [P, KT * D], BF, kind="ExternalInput")
    bq_d = nc.dram_tensor("bq", [P, NPAIR], F32, kind="ExternalInput")
    bk_d = nc.dram_tensor("bk", [P, NPAIR], F32, kind="ExternalInput")
    bv_d = nc.dram_tensor("bv", [1, D], BF, kind="ExternalInput")
    bp_d = nc.dram_tensor("bp", [1, D], BF, kind="ExternalInput")
    out_d = nc.dram_tensor("out", [TOK, D], BF, kind="ExternalOutput")

    with tile.TileContext(nc) as tc:
        with (
            tc.tile_pool(name="sing", bufs=1) as sing,
            tc.tile_pool(name="psS", bufs=2, space="PSUM") as psS,
            tc.tile_pool(name="psO", bufs=4, space="PSUM") as psO,
            tc.tile_pool(name="ebuf", bufs=4) as ebuf,
            tc.tile_pool(name="qkp", bufs=2) as qkp,
            tc.tile_pool(name="wqkp", bufs=2) as wqkp,
            tc.tile_pool(name="rbuf", bufs=2) as rbuf,
            tc.tile_pool(name="outp", bufs=2) as outp,
        ):
            from concourse import library_config

            nc.gpsimd.load_library(library_config.attn)

            # ---------------- persistent SBUF ----------------
            # DMA priority order: x + wv first (the V phase is the
            # critical path to the first matmuls)
            x_sb = sing.tile([P, KT, TOK], F16, tag="x")
            half = KT // 2
            nc.sync.dma_start(
                out=x_sb[:, 0:half, :], in_=x_d[:, 0 : half * TOK]
            )
            nc.sync.dma_start(
                out=x_sb[:, half:KT, :], in_=x_d[:, half * TOK : KT * TOK]
            )
            x3 = x_sb
            wv_sb = sing.tile([P, KT, D], F16, tag="wv")
            nc.sync.dma_start(out=wv_sb, in_=wv_d[:, :])

            # memset cannot target f32r/bf16; stage in f32, cast via DVE
            cst_sb = sing.tile([2, P], F32, tag="cst")
            nc.vector.memset(cst_sb, 1.0)
            ones_bf = sing.tile([1, P], BF, tag="ones")
            nc.vector.tensor_copy(out=ones_bf, in_=cst_sb[0:1, :])
            vones_sb = sing.tile([P, MT * H], F32, tag="vones")
            nc.vector.memset(vones_sb, AUG)

            bq_sb = sing.tile([P, NPAIR], F32, tag="bq")
            nc.sync.dma_start(out=bq_sb, in_=bq_d[:, :])
            bk_sb = sing.tile([P, NPAIR], F32, tag="bk")
            nc.sync.dma_start(out=bk_sb, in_=bk_d[:, :])
            bv_sb = sing.tile([1, D], BF, tag="bv")
            nc.sync.dma_start(out=bv_sb, in_=bv_d[:, :])
            bp_sb = sing.tile([1, D], BF, tag="bp")
            nc.sync.dma_start(out=bp_sb, in_=bp_d[:, :])

            v_sb = sing.tile([P, MT, VW], BF, tag="v")   # v_aug
            # aug columns (denominator accumulators) = 1/att_scale
            nc.vector.tensor_copy(
                out=v_sb[:, :, :]
                .rearrange("p m (h e) -> p m h e", e=DH + 1)[:, :, :, DH],
                in_=vones_sb[:, :].rearrange("p (m h) -> p m h", h=H),
            )
            ao_sb = sing.tile([P, NPAIR, TOK], BF, tag="ao")  # attout^T

            wq3 = wq_d[:, :].rearrange("p (pr kt n) -> p pr kt n", pr=NPAIR, kt=KT)
            wk3 = wk_d[:, :].rearrange("p (pr kt n) -> p pr kt n", pr=NPAIR, kt=KT)

            # ---------------- Q/K projection task ----------------
            # qT/kT for one pair: [P rows = (even|odd head dims), TOK]
            qT = {}
            kT = {}

            def load_wqk(p):
                wq_sb = wqkp.tile([P, KT, P], F16, tag="wq")
                nc.sync.dma_start(
                    out=wq_sb, in_=wq3[:, p, :, :]
                )
                wk_sb = wqkp.tile([P, KT, P], F16, tag="wk")
                nc.sync.dma_start(
                    out=wk_sb, in_=wk3[:, p, :, :]
                )
                return wq_sb, wk_sb

            def emit_qk_task(p, which, w_sb):
                """One accumulation task: (x @ w_pair)^T full width + bias."""
                if which == "q":
                    if p not in qT:
                        qT[p] = qkp.tile([P, TOK], F16, tag="qT", name=f"qT{p}")
                    dst, b_sb = qT[p], bq_sb
                else:
                    if p not in kT:
                        kT[p] = qkp.tile([P, TOK], F16, tag="kT", name=f"kT{p}")
                    dst, b_sb = kT[p], bk_sb
                for t0 in range(0, TOK, QCH):
                    tw = min(QCH, TOK - t0)
                    ps = psS.tile([P, 2 * QCH], F32, tag="S", name="ps_qk")
                    for kt in range(KT):
                        nc.tensor.matmul(
                            ps[:, 0:tw],
                            lhsT=w_sb[:, kt, :],
                            rhs=x3[:, kt, t0 : t0 + tw],
                            start=(kt == 0),
                            stop=(kt == KT - 1),
                        )
                    nc.vector.tensor_scalar_add(
                        out=dst[:, t0 : t0 + tw],
                        in0=ps[:, 0:tw],
                        scalar1=b_sb[:, p : p + 1],
                    )

            # ---------------- V phase: v_aug = [x wv + bv | AUG] --------
            wp_sb = sing.tile([P, KT, D], BF, tag="wp")
            wqk0 = load_wqk(0)
            WCH = min(QCH, D)       # matmul out <= 1 PSUM bank
            VWCH = min(QCH, D)      # V stream width (fp16 max 512)
            for c in range(0, D, VWCH):
                for mt in range(MT):
                    ps_v = psS.tile([P, 2 * QCH], F32, tag="S", name="ps_v")
                    for kt in range(KT):
                        nc.tensor.matmul(
                            ps_v[:, 0:VWCH],
                            lhsT=x3[:, kt, mt * P : (mt + 1) * P],
                            rhs=wv_sb[:, kt, c : c + VWCH],
                            start=(kt == 0),
                            stop=False,
                        )
                    nc.tensor.matmul(
                        ps_v[:, 0:VWCH],
                        lhsT=ones_bf[0:1, 0:P],
                        rhs=bv_sb[0:1, c : c + VWCH],
                        start=False,
                        stop=True,
                    )
                    # scatter heads into v_aug (DH+1 stride)
                    nh = VWCH // DH
                    h0 = c // DH
                    nc.vector.tensor_copy(
                        out=v_sb[:, mt, :]
                        .rearrange("p (h e) -> p h e", e=DH + 1)[
                            :, h0 : h0 + nh, 0:DH
                        ],
                        in_=ps_v[:, 0:VWCH].rearrange(
                            "p (h d) -> p h d", d=DH
                        ),
                    )
                # interleave pair-0 Q/K projections into the late V
                # phase (behind the x/wv DMA)
                if c + VWCH >= D:
                    emit_qk_task(0, "q", wqk0[0])
                    emit_qk_task(0, "k", wqk0[1])

            nc.sync.dma_start(out=wp_sb, in_=wp_d[:, :])

            # ---------------- attention, per head pair ----------------
            # Software-pipelined conveyor: per 2-kb group emit scores,
            # then exp, then the PREVIOUS group's att@V (so the PE queue
            # never sits behind an exp that hasn't finished). The
            # normalize for each (pair, qh) is split: the DVE reciprocal
            # chain is emitted at block end, but the PE broadcast + final
            # muls are deferred into the next block.
            pending_norm = []

            def flush_norm():
                if not pending_norm:
                    return
                p_, q0_, ps_oe_, ps_oo_, rc_ = pending_norm.pop(0)
                # partition-broadcast of both heads' reciprocals (GPSIMD,
                # keeps the PE out of the normalize chain entirely)
                bcs = rbuf.tile([DH, 2 * QCH], BF, tag="bcs")
                nc.gpsimd.partition_broadcast(
                    out_ap=bcs, in_ap=rc_[0:1, :], channels=DH
                )
                nc.vector.tensor_mul(
                    out=ao_sb[0:DH, p_, q0_ : q0_ + QCH],
                    in0=ps_oe_[0:DH, :],
                    in1=bcs[0:DH, 0:QCH],
                )
                nc.vector.tensor_mul(
                    out=ao_sb[DH:P, p_, q0_ : q0_ + QCH],
                    in0=ps_oo_[0:DH, :],
                    in1=bcs[0:DH, QCH : 2 * QCH],
                )

            for p in range(NPAIR):
                wqk_next = load_wqk(p + 1) if p + 1 < NPAIR else None
                # one continuous conveyor over all (qh, kb) units of the
                # pair -- no drain between the q-halves
                units = [(qh, kb) for qh in range(NQH) for kb in range(MT)]
                NU = len(units)
                ps_o = {}

                def get_o(qh):
                    if qh not in ps_o:
                        oe = psO.tile([P, QCH], F32, tag="O", name="ps_oe")
                        oo = psO.tile([P, QCH], F32, tag="O", name="ps_oo")
                        ps_o[qh] = (oe, oo)
                    return ps_o[qh]

                def emit_attv(us, e_ts):
                    for (qh, kb), et in zip(us, e_ts):
                        for hoff, ps_out in ((0, get_o(qh)[0]),
                                             (1, get_o(qh)[1])):
                            hh = 2 * p + hoff
                            nc.tensor.matmul(
                                ps_out[0 : DH + 1, :],
                                lhsT=v_sb[
                                    :, kb,
                                    hh * (DH + 1) : (hh + 1) * (DH + 1),
                                ],
                                rhs=et[:, hoff * QCH : (hoff + 1) * QCH],
                                start=(kb == 0),
                                stop=(kb == MT - 1),
                                skip_group_check=True,
                            )

                def emit_norm_chain(qh):
                    ps_oe, ps_oo = get_o(qh)
                    rq = rbuf.tile([1, 2 * QCH], F32, tag="rq")
                    nc.vector.tensor_copy(
                        out=rq[0:1, 0:QCH], in_=ps_oe[DH : DH + 1, :]
                    )
                    nc.vector.tensor_copy(
                        out=rq[0:1, QCH : 2 * QCH],
                        in_=ps_oo[DH : DH + 1, :],
                    )
                    rr = rbuf.tile([1, 2 * QCH], F32, tag="rr")
                    nc.vector.reciprocal_approx_fast(out=rr, in_=rq)
                    rc = rbuf.tile([1, 2 * QCH], BF, tag="rc")
                    nc.vector.tensor_copy(out=rc, in_=rr)
                    pending_norm.append((p, qh * QCH, ps_oe, ps_oo, rc))

                gq = (NU // 2) & ~1
                gk = max(0, NU - 4)
                prev = None
                for g in range(0, NU, 2):
                    us = units[g : g + 2]
                    e_ts = []
                    for qh, kb in us:
                        q0 = qh * QCH
                        st = psS.tile([P, 2 * QCH], F32, tag="S")
                        # even head: PE tile T0 (SBUF rows 0:64)
                        # odd head: T8 (rows 64:128) -- concurrent
                        nc.tensor.matmul(
                            st[:, 0:QCH],
                            lhsT=kT[p][0:DH, kb * P : (kb + 1) * P],
                            rhs=qT[p][0:DH, q0 : q0 + QCH],
                            start=True,
                            stop=True,
                        )
                        nc.tensor.matmul(
                            st[:, QCH : 2 * QCH],
                            lhsT=kT[p][DH:P, kb * P : (kb + 1) * P],
                            rhs=qT[p][DH:P, q0 : q0 + QCH],
                            start=True,
                            stop=True,
                        )
                        et = ebuf.tile([P, 2 * QCH], BF, tag="E")
                        nc.scalar.activation(out=et, in_=st, func=EXP)
                        e_ts.append(et)
                    if g == 0:
                        # previous pair's deferred qh1 broadcast+muls
                        flush_norm()
                    if g == MT + 4:
                        # this pair's qh0 broadcast+muls
                        flush_norm()
                    if prev is not None:
                        emit_attv(*prev)
                        if prev[0][-1][1] == MT - 1:
                            emit_norm_chain(prev[0][-1][0])
                    if wqk_next is not None:
                        if g == gq:
                            emit_qk_task(p + 1, "q", wqk_next[0])
                        if g == gk:
                            emit_qk_task(p + 1, "k", wqk_next[1])
                    prev = (us, e_ts)
                emit_attv(*prev)
                if prev[0][-1][1] == MT - 1:
                    emit_norm_chain(prev[0][-1][0])
            while pending_norm:
                flush_norm()

            # ---------------- projection: out = ao^T.T wp + bp -------
            # kt order is pair-completion order, so the scheduler can
            # run the first KT-1 accumulation steps of each output tile
            # during the last pair's attention.
            for c in range(0, D, WCH):
                for mt in range(MT):
                    ps_p = psS.tile([P, 2 * QCH], F32, tag="S", name="ps_p")
                    for kt in range(KT):
                        nc.tensor.matmul(
                            ps_p[:, 0:WCH],
                            lhsT=ao_sb[:, kt, mt * P : (mt + 1) * P],
                            rhs=wp_sb[:, kt, c : c + WCH],
                            start=(kt == 0),
                            stop=False,
                        )
                    nc.tensor.matmul(
                        ps_p[:, 0:WCH],
                        lhsT=ones_bf[0:1, 0:P],
                        rhs=bp_sb[0:1, c : c + WCH],
                        start=False,
                        stop=True,
                    )
                    o_sb = outp.tile([P, WCH], BF, tag="o")
                    nc.vector.tensor_copy(
                        out=o_sb[:, 0:WCH], in_=ps_p[:, 0:WCH]
                    )
                    nc.sync.dma_start(
                        out=out_d[mt * P : (mt + 1) * P, c : c + WCH],
                        in_=o_sb[:, 0:WCH],
                    )

    return nc


# ---------------------------------------------------------------------------
# host-side layout prep
# ---------------------------------------------------------------------------

def _round_f32r(x):
    """RNE to f32r's 11-explicit-mantissa-bit grid (matches HW rounding)."""
    u = np.ascontiguousarray(x, np.float32).view(np.uint32)
    u = ((u + np.uint32(1 << 11)) >> 12) << 12
    return u.view(np.float32)


def _tile_rows(w):
    """[D, N] -> [P, (D//P) * N] with [p][kt][n] layout."""
    Dd, N = w.shape
    KT = Dd // P
    return np.ascontiguousarray(
        w.reshape(KT, P, N).transpose(1, 0, 2).reshape(P, KT * N)
    )


def host_prep_shared(w_qkv, b_qkv, w_proj, b_proj, D, H):
    """Split/retile the weights once for all cores."""
    NPAIR = H // 2
    VCH = min(512, D)
    NVCH = D // VCH

    wq3 = w_qkv.reshape(D, H, DH, 3)
    wq = np.ascontiguousarray(wq3[:, :, :, 0].reshape(D, D))
    wk = np.ascontiguousarray(wq3[:, :, :, 1].reshape(D, D))
    wv = np.ascontiguousarray(wq3[:, :, :, 2].reshape(D, D))
    wp = np.ascontiguousarray(np.asarray(w_proj, np.float32))

    NPBF = mybir.dt.np(mybir.dt.bfloat16)

    def pair_major(w):  # [D, D] -> [P, NPAIR*KT*P] fp16, pair-block major
        blocks = [
            _tile_rows(w[:, p * P : (p + 1) * P]) for p in range(NPAIR)
        ]
        return np.concatenate(blocks, axis=1).astype(np.float16)

    out = {
        "wq": pair_major(wq),
        "wk": pair_major(wk),
        "wv": _tile_rows(wv).astype(np.float16),
        "wp": _tile_rows(wp).astype(NPBF),
    }
    b3 = np.asarray(b_qkv, np.float32).reshape(H, DH, 3)
    bq = np.ascontiguousarray(b3[:, :, 0].reshape(D))
    bk = np.ascontiguousarray(b3[:, :, 1].reshape(D))
    bv = np.ascontiguousarray(b3[:, :, 2].reshape(D))
    out["bq"] = np.ascontiguousarray(bq.reshape(NPAIR, P).T).astype(np.float32)
    out["bk"] = np.ascontiguousarray(bk.reshape(NPAIR, P).T).astype(np.float32)
    out["bv"] = bv.reshape(1, D).astype(NPBF)
    out["bp"] = np.asarray(b_proj, np.float32).reshape(1, D).astype(NPBF)
    return out


def host_prep_x(x_b, TOK, D):
    """One batch element [TOK, D] -> {x: f32r, xv: bf16} tiled [P, KT*TOK]."""
    xT = np.ascontiguousarray(np.asarray(x_b, np.float32).T)  # [D, TOK]
    return {"x": _tile_rows(xT).astype(np.float16)}


# ---------------------------------------------------------------------------
# entry point
# ---------------------------------------------------------------------------

_BUILT = {}


def _get_nc(TOK, D, H, att_scale):
    key = (TOK, D, H, att_scale)
    if key not in _BUILT:
        nc = bacc.Bacc(
            "TRN2",
            target_bir_lowering=False,
            debug=False,
            dynamic_dma_scratch_size=512,
        )
        build(nc, TOK, D, H, att_scale)
        nc.compile()
        nc.finalize()
        _BUILT[key] = nc
    return _BUILT[key]


def kernel(x, w_qkv, b_qkv, w_proj, b_proj):
    from concourse.bass_utils import run_bass_kernel_spmd

    x = np.asarray(x, np.float32)
    B, TOK, D = x.shape
    H = H_FULL
    shared = host_prep_shared(
        np.asarray(w_qkv, np.float32),
        np.asarray(b_qkv, np.float32),
        np.asarray(w_proj, np.float32),
        np.asarray(b_proj, np.float32),
        D,
        H,
    )
    in_maps = []
    for b in range(B):
        m = dict(shared)
        m.update(host_prep_x(x[b], TOK, D))
        in_maps.append(m)

    nc = _get_nc(TOK, D, H, ATT_SCALE_FULL)
    res = run_bass_kernel_spmd(nc, in_maps, list(range(N_CORES)))
    out = np.stack([res.results[b]["out"] for b in range(B)], axis=0)
    return out.astype(np.float32)
